# revision 18
# baseline (speedup 1.0000x reference)
"""Fused Conv3d + per-batch global stats kernel for Trainium2 (8 NeuronCores).

Problem: x [16,64,32,32,32] f32, conv_weight [128,64,3,3,3], conv_bias [128].
  y = conv3d(x, w, VALID) + b        -> [16,128,30,30,30]
  out[n] = mean_n / sqrt(var_n + eps) over (C,D,H,W)   -> [16] f32

Strategy:
  - Data parallel: batch 16 -> 8 cores x 2 batches, weights replicated.
  - Conv as 27 tap-matmuls contracting Cin=64, accumulated in PSUM.
    y never materialized in HBM: only per-channel sum / sum-of-squares
    (bias folded analytically at the end).
  - K=64 matmuls packed 2x via PE row tiling: taps 0..13 on array rows
    0-63 (tile_position (0,0), PSUM bank A), taps 14..26 on rows 64-127
    (tile_position (64,0), PSUM bank B). x is duplicated on SBUF
    partitions 64-127 so each half streams independently. Banks are
    combined during the stats reduction (row tiles must not share a
    PSUM bank).
  - float32r matmul datapath (1 cycle/row at N>=256, ~1.5e-4 rel
    accuracy) with an on-device DVE rounding pass fp32 -> f32r.
  - Per-od output rows in 2 PSUM chunks (510+390); garbage columns
    excluded via strided APs in the matmul rhs so PSUM is densely
    packed with valid positions.
  - Stats per chunk: one DVE tensor_tensor_reduce (ym = psA + psB with
    fused row-sum into a per-chunk scratch column) + one ScalarE
    Square pass with fused accumulation into a second scratch column.
    Scratch columns are reduced once per batch -- no serial S/S2
    accumulator chain.
  - Head: plane DMAs issued before the weight DMAs; weights arrive in
    4 quarter-tiles with pipelined f32r rounding; the PE is prewarmed
    on memset junk from ~7us so the HAM clock gate is released before
    real matmuls start.
  - Tail: cross-partition reduce via GpSimd partition_all_reduce (no
    SBUF->SBUF DMA hop), short mostly-DVE finalize chain, one combined
    out DMA for both batches.
"""
import os
os.environ.setdefault("NEURON_RT_RESET_CORES", "1")

import numpy as np
from contextlib import ExitStack

import concourse.bass as bass
import concourse.tile as tile
from concourse import bacc, bass_isa, mybir
from concourse.bass_utils import run_bass_kernel_spmd

N_CORES = 8
CIN, COUT, KK = 64, 128, 3
D = H = W = 32
PL = H * W                      # 1024 linear positions per D-plane
OD = OH = OW = 30
NPOS = OD * OH * OW             # 27000 valid positions per (n, c)
NTOT = COUT * NPOS
EPS = 1e-5
NB = 2                          # batches per core
TAPS = [(kd, kh, kw) for kd in range(KK) for kh in range(KK) for kw in range(KK)]
# Two tap splits, alternated per chunk so each row tile averages 13.5
# matmuls (27 taps can't split evenly into 2x row tiles).
SPLITS = [(TAPS[:14], TAPS[14:]),   # 14 on T0, 13 on T8
          (TAPS[:13], TAPS[13:])]   # 13 on T0, 14 on T8
# (row0, n_valid_cols, n_oh_rows): matmul rhs streams only the 30 valid
# ow columns per oh row via a 3D strided AP, so PSUM is densely packed
# with valid positions (900 streamed cols/plane instead of 958).
CHUNKS = [(0, 510, 17), (17, 390, 13)]
# last od uses 3 smaller chunks (all >=256 cols for the full-rate f32r
# path) so the terminal stats chain is shorter
CHUNKS_LAST = [(0, 300, 10), (10, 300, 10), (20, 300, 10)]
NCHUNK = (OD - 1) * len(CHUNKS) + len(CHUNKS_LAST)
QT = 7 * 128                    # weight-quarter width (7 taps)

F32 = mybir.dt.float32
F32R = mybir.dt.float32r
ADD = mybir.AluOpType.add
MULT = mybir.AluOpType.mult
SUB = mybir.AluOpType.subtract


def _emit(nc):
    x_ap = nc.dram_tensor("x", [NB, CIN, D * PL], F32, kind="ExternalInput").ap()
    wq_ap = nc.dram_tensor("wq", [128, 2 * 14 * 128], F32, kind="ExternalInput").ap()
    b_ap = nc.dram_tensor("bias", [128, 1], F32, kind="ExternalInput").ap()
    out_ap = nc.dram_tensor("out", [1, NB], F32, kind="ExternalOutput").ap()

    with tile.TileContext(nc) as tc, ExitStack() as ctx:
        wpool = ctx.enter_context(tc.tile_pool(name="w", bufs=1))
        cpool = ctx.enter_context(tc.tile_pool(name="const", bufs=1))
        stpool = ctx.enter_context(tc.tile_pool(name="stage", bufs=4))
        xgpool = ctx.enter_context(tc.tile_pool(name="xg", bufs=7))
        pspool = ctx.enter_context(tc.tile_pool(name="ps", bufs=8, space="PSUM"))
        aspool = ctx.enter_context(tc.tile_pool(name="as", bufs=4))
        ympool = ctx.enter_context(tc.tile_pool(name="ym", bufs=4))
        y2pool = ctx.enter_context(tc.tile_pool(name="y2", bufs=4))
        accpool = ctx.enter_context(tc.tile_pool(name="acc", bufs=2))
        finpool = ctx.enter_context(tc.tile_pool(name="fin", bufs=1))

        # --- PE prewarm on junk: HAM runs the PE cold (half clock) until
        # ~3.4us of sustained matmul activity. Burn the ramp on memset
        # data so it starts right after the framework preamble, during
        # the DMA head, instead of after the first real operands land.
        junks = cpool.tile([128, 512], F32, tag="junks")
        junkx = cpool.tile([128, 512], F32R, tag="junkx")
        junkw = cpool.tile([128, 128], F32R, tag="junkw")
        nc.vector.memset(junks[:, :], 0.0)
        nc.vector.tensor_copy(junkx[:, :], junks[:, :])
        nc.vector.tensor_copy(junkw[:, :], junks[:, 0:128])
        pwA = pspool.tile([128, 512], F32, tag="ps")
        pwB = pspool.tile([128, 512], F32, tag="ps")
        for i in range(8):
            nc.tensor.matmul(pwA[:, 0:512], junkw[0:64, :], junkx[0:64, :],
                             start=(i == 0), stop=(i == 7), tile_position=(0, 0))
            nc.tensor.matmul(pwB[:, 0:512], junkw[64:128, :], junkx[64:128, :],
                             start=(i == 0), stop=(i == 7), tile_position=(64, 0))

        # --- one-time loads. Planes 0-2 are issued before the weights so
        # the first-plane cast isn't queued behind the 1.8MB weight DMA.
        stage = {}

        def load_plane_dma(b, p):
            st = stpool.tile([128, PL], F32, tag="st")
            src = x_ap[b][:, p * PL:(p + 1) * PL]
            nc.sync.dma_start(st[0:64, :], src)
            nc.sync.dma_start(st[64:128, :], src)
            stage[(b, p)] = st

        # weights in 4 quarter-tiles (7 taps each) with pipelined f32r
        # rounding on the otherwise-idle ScalarE; the first chunk's
        # matmuls only wait for the quarters they read. Plane and weight
        # DMA issues are interleaved so neither gates the other.
        wqr = [wpool.tile([128, QT], F32R, tag=f"wqr{q}", name=f"wqr{q}")
               for q in range(4)]

        def load_wq(q):
            wq = wpool.tile([128, QT], F32, tag=f"wq{q}")
            nc.sync.dma_start(wq[:, :], wq_ap[:, q * QT:(q + 1) * QT])
            nc.scalar.copy(wqr[q][:, :], wq[:, :])    # round to f32r

        load_wq(0)
        load_plane_dma(0, 0)
        load_wq(1)
        load_plane_dma(0, 1)
        load_plane_dma(0, 2)
        load_wq(2)
        load_wq(3)

        def wslice(split, half, i):
            # weight AP for tap i of row-half `half` in split `split`
            q, col = divmod(split * 14 + i, 7)
            rows = slice(0, 64) if half == 0 else slice(64, 128)
            return wqr[q][rows, col * 128:(col + 1) * 128]

        bias_t = cpool.tile([128, 1], F32, tag="bias")
        nc.sync.dma_start(bias_t[:, :], b_ap[:, :])
        negeps_t = cpool.tile([1, 1], F32, tag="negeps")
        nc.vector.memset(negeps_t[:, :], -EPS * float(NTOT) * float(NTOT))
        ones_t = cpool.tile([128, 1], F32, tag="ones")
        nc.vector.memset(ones_t[:, :], 1.0)

        # batch-independent bias reductions: cb = sum_c(b), sum_c(b^2).
        # Cross-partition reduce: flatten [128,2] -> [1,256] via
        # SBUF-to-SBUF DMA, then DVE-reduce the 128-long stride.
        cbin = cpool.tile([128, 2], F32, tag="cbin")
        nc.vector.tensor_copy(cbin[:, 0:1], bias_t[:, 0:1])
        nc.vector.tensor_mul(cbin[:, 1:2], bias_t[:, 0:1], bias_t[:, 0:1])
        cbcat = cpool.tile([1, 256], F32, tag="cbcat")
        nc.sync.dma_start(cbcat[0:1, 0:256], cbin[:, 0:2])
        cb = cpool.tile([1, 2], F32, tag="cb")
        nc.vector.tensor_reduce(
            cb[0:1, 0:2],
            cbcat[0:1, 0:256].rearrange("p (a b) -> p b a", b=2),
            axis=mybir.AxisListType.X, op=ADD)

        resb = finpool.tile([1, NB], F32, tag="resb")

        for b in range(NB):
            sscr = accpool.tile([128, NCHUNK], F32, tag="sscr")
            s2scr = accpool.tile([128, NCHUNK], F32, tag="s2scr")

            xp = {}

            def load_plane(p, b=b):
                if p in xp or p >= D:
                    return
                if (b, p) in stage:
                    st = stage.pop((b, p))
                else:
                    st = stpool.tile([128, PL], F32, tag="st")
                    src = x_ap[b][:, p * PL:(p + 1) * PL]
                    nc.sync.dma_start(st[0:64, :], src)
                    nc.sync.dma_start(st[64:128, :], src)
                # +32 col slack: the strided rhs slice of the last oh row
                # spans past PL (its b>=30 tail is never addressed)
                t = xgpool.tile([128, PL + 32], F32R, tag="xg")
                nc.vector.tensor_copy(t[0:64, 0:PL], st[0:64, :])   # round to f32r
                nc.vector.tensor_copy(t[64:128, 0:PL], st[64:128, :])
                xp[p] = t

            for p in range(3):
                load_plane(p)

            chunk_idx = 0
            for od in range(OD):
                load_plane(od + 3)
                load_plane(od + 4)
                for g in [g for g in xp if g < od]:
                    del xp[g]

                # Cross-chunk slot interleave: T0's and T8's tap streams
                # for ALL of this od's chunks are emitted as two aligned
                # queues, so the lone unpaired slot of an (n,n-1) split
                # in one chunk overlaps the lone slot of the next chunk
                # (27 effective pair-slots per od instead of 28).
                t0q, t8q, specs = [], [], []
                for (r0, NC, NROW) in (CHUNKS if od < OD - 1 else CHUNKS_LAST):
                    split = chunk_idx % 2
                    ta, tb = SPLITS[split]
                    ci = chunk_idx
                    chunk_idx += 1
                    psA = pspool.tile([128, 512], F32, tag="ps")
                    psB = pspool.tile([128, 512], F32, tag="ps")
                    specs.append((psA, psB, NC, ci))
                    for i, tap in enumerate(ta):
                        t0q.append((split, i, tap, psA, r0, NC, NROW, len(ta)))
                    for i, tap in enumerate(tb):
                        t8q.append((split, i, tap, psB, r0, NC, NROW, len(tb)))

                for k in range(max(len(t0q), len(t8q))):
                    for half, q in ((0, t0q), (1, t8q)):
                        if k >= len(q):
                            continue
                        split, i, (kd, kh, kw), ps, r0, NC, NROW, n = q[k]
                        off = kh * W + kw + r0 * W
                        rows = slice(0, 64) if half == 0 else slice(64, 128)
                        nc.tensor.matmul(
                            ps[:, 0:NC],
                            wslice(split, half, i),
                            xp[od + kd][rows, off:off + NROW * W].rearrange(
                                "p (a b) -> p a b", b=W)[:, :, 0:OW],
                            start=(i == 0), stop=(i == n - 1),
                            tile_position=(0, 0) if half == 0 else (64, 0))

                for (psA, psB, NC, ci) in specs:
                    # stats: ym = psA + psB, row-sum into the chunk's
                    # scratch column; Square accumulates the row-sum of
                    # ym^2 into the second scratch. (The DVE can read
                    # only one PSUM operand per instruction, so ScalarE
                    # stages psA into SBUF first.)
                    aS = aspool.tile([128, 512], F32, tag="aS")
                    nc.scalar.copy(aS[:, 0:NC], psA[:, 0:NC])
                    ym = ympool.tile([128, 512], F32, tag="ym")
                    nc.vector.tensor_add(ym[:, 0:NC], aS[:, 0:NC], psB[:, 0:NC])
                    nc.vector.tensor_reduce(sscr[:, ci:ci + 1], ym[:, 0:NC],
                                            axis=mybir.AxisListType.X, op=ADD)
                    sq = y2pool.tile([128, 512], F32, tag="sq")
                    nc.scalar.activation(sq[:, 0:NC], ym[:, 0:NC],
                                         mybir.ActivationFunctionType.Square,
                                         accum_out=s2scr[:, ci:ci + 1])

            # --- finalize batch ---
            # reduce scratch columns, fold bias per channel, then one
            # GpSimd cross-partition all-reduce:
            #   T1 = sum_c S + NPOS * sum_c b
            #   T2 = sum_c S2 + 2 * sum_c (b*S) + NPOS * sum_c b^2
            #   out = T1 / sqrt(NTOT*T2 - T1^2 + eps*NTOT^2)
            packed = accpool.tile([128, 3], F32, tag="packed")
            nc.vector.tensor_reduce(packed[:, 0:1], sscr[:, :],
                                    axis=mybir.AxisListType.X, op=ADD)
            nc.vector.tensor_reduce(packed[:, 1:2], s2scr[:, :],
                                    axis=mybir.AxisListType.X, op=ADD)
            nc.vector.tensor_mul(packed[:, 2:3], packed[:, 0:1], bias_t[:, 0:1])
            red = accpool.tile([1, 3], F32, tag="red3")
            cat = accpool.tile([1, 384], F32, tag="cat")
            nc.sync.dma_start(cat[0:1, 0:384], packed[:, 0:3])
            nc.vector.tensor_reduce(
                red[0:1, 0:3],
                cat[0:1, 0:384].rearrange("p (a b) -> p b a", b=3),
                axis=mybir.AxisListType.X, op=ADD)

            f = finpool.tile([1, 8], F32, tag=f"fin{b}")
            # T1 = NPOS*cb0 + redS
            nc.vector.scalar_tensor_tensor(
                f[0:1, 0:1], cb[0:1, 0:1], float(NPOS), red[0:1, 0:1],
                op0=MULT, op1=ADD)
            # t2a = 2*red_bS + redS2 ; T2 = NPOS*cb1 + t2a
            nc.vector.scalar_tensor_tensor(
                f[0:1, 1:2], red[0:1, 2:3], 2.0, red[0:1, 1:2],
                op0=MULT, op1=ADD)
            nc.vector.scalar_tensor_tensor(
                f[0:1, 2:3], cb[0:1, 1:2], float(NPOS), f[0:1, 1:2],
                op0=MULT, op1=ADD)
            # m1 = T1*T1 - eps*NTOT^2 ; d = NTOT*T2 - m1
            nc.vector.scalar_tensor_tensor(
                f[0:1, 3:4], f[0:1, 0:1], f[0:1, 0:1], negeps_t[0:1, 0:1],
                op0=MULT, op1=ADD)
            nc.vector.scalar_tensor_tensor(
                f[0:1, 4:5], f[0:1, 2:3], float(NTOT), f[0:1, 3:4],
                op0=MULT, op1=SUB)
            nc.scalar.activation(f[0:1, 5:6], f[0:1, 4:5],
                                 mybir.ActivationFunctionType.Sqrt)
            nc.vector.reciprocal(f[0:1, 6:7], f[0:1, 5:6])
            nc.vector.tensor_mul(resb[0:1, b:b + 1], f[0:1, 0:1], f[0:1, 6:7])

        nc.sync.dma_start(out_ap[0:1, 0:NB], resb[0:1, 0:NB])


_NC_CACHE = None


def _module():
    global _NC_CACHE
    if _NC_CACHE is None:
        nc = bacc.Bacc("TRN2", target_bir_lowering=False, debug=False,
                       num_devices=N_CORES)
        _emit(nc)
        nc.compile()
        _NC_CACHE = nc
    return _NC_CACHE


def _prep_weights(conv_weight):
    wq = np.zeros((128, 2 * 14 * 128), dtype=np.float32)
    for s, (ta, tb) in enumerate(SPLITS):
        woff = s * 14 * 128
        for i, (kd, kh, kw) in enumerate(ta):
            wq[0:64, woff + i * 128:woff + (i + 1) * 128] = \
                conv_weight[:, :, kd, kh, kw].T
        for i, (kd, kh, kw) in enumerate(tb):
            wq[64:128, woff + i * 128:woff + (i + 1) * 128] = \
                conv_weight[:, :, kd, kh, kw].T
    return wq


def kernel(x, conv_weight, conv_bias):
    x = np.ascontiguousarray(np.asarray(x, dtype=np.float32))
    w = np.asarray(conv_weight, dtype=np.float32)
    bias = np.asarray(conv_bias, dtype=np.float32)

    wq = _prep_weights(w)
    bias2 = np.ascontiguousarray(bias.reshape(128, 1))
    xr = x.reshape(16, CIN, D * PL)

    in_maps = []
    for c in range(N_CORES):
        in_maps.append({
            "x": np.ascontiguousarray(xr[NB * c:NB * (c + 1)]),
            "wq": wq,
            "bias": bias2,
        })

    nc = _module()
    res = run_bass_kernel_spmd(nc, in_maps, core_ids=list(range(N_CORES)))

    out = np.empty(16, dtype=np.float32)
    for c in range(N_CORES):
        out[NB * c:NB * (c + 1)] = res.results[c]["out"].reshape(NB)
    return out
